# revision 13
# baseline (speedup 1.0000x reference)
"""MAGRU forward on 8 Trainium2 NeuronCores.

Strategy: data-parallel over N (8 envs/core). Within a core, the scan over
T=512 decomposes into independent "chains" delimited by reset=1 positions
(Bernoulli 0.5 -> ~2048 chains, max depth ~17), because at a reset position
h_prev is the learned constant initial_h. Waves over chain depth:
  wave 0 = all chain starts (h_prev = initial_h, or carry[n,0] for the few
           t=0 non-reset chains), action-sorted -> maskless matmuls.
  wave k = depth-k positions; h_prev = contiguous prefix segments of wave
           k-1's output buffer (chains sorted by length desc inside each
           first-action group). Per-action weight selection via masked-rhs
           PSUM accumulation; 0/1 masks are host-provided data.
SPMD: one instruction stream for all 8 cores; per-core chain-length
multisets are padded to the elementwise max so the schedule is shared.
"""

import numpy as np

N, T, D, H, A = 64, 512, 128, 128, 6
G3 = 3 * H  # 384
NCORES = 8
NLOC = N // NCORES  # 8 envs per core
PSUM_TILE = 512  # max free dim of one PSUM f32 bank


# ----------------------------------------------------------------------------
# Host-side schedule
# ----------------------------------------------------------------------------

class Chain:
    __slots__ = ("n", "t0", "L", "is_delta", "dummy")

    def __init__(self, n, t0, L, is_delta, dummy=False):
        self.n = n
        self.t0 = t0
        self.L = L
        self.is_delta = is_delta  # t0 == 0 and not reset -> h_prev = carry[n,0]
        self.dummy = dummy


def _core_chains(reset_core):
    """reset_core: [NLOC, T] bool. Returns list of Chain (real only)."""
    chains = []
    for n in range(NLOC):
        starts = [0] + [t for t in range(1, T) if reset_core[n, t]]
        starts_set = starts + [T]
        for i, t0 in enumerate(starts):
            L = starts_set[i + 1] - t0
            is_delta = (t0 == 0) and (not reset_core[n, 0])
            chains.append(Chain(n, t0, L, is_delta))
    return chains


class Schedule:
    """Common (padded) schedule shared by all cores + per-core chain lists."""

    def __init__(self, reset, a):
        # group: 6 = delta group (t0==0, no reset); 0..5 = alpha0 groups
        per_core = []
        for c in range(NCORES):
            chains = _core_chains(reset[c * NLOC:(c + 1) * NLOC])
            groups = [[] for _ in range(7)]
            for ch in chains:
                g = 6 if ch.is_delta else int(a[c * NLOC + ch.n, ch.t0])
                groups[g].append(ch)
            per_core.append(groups)

        # common per-(group, length) counts = max over cores
        maxlen = max(ch.L for groups in per_core for g in range(7) for ch in groups[g])
        cnt = np.zeros((7, maxlen + 1), dtype=np.int64)
        for groups in per_core:
            local = np.zeros_like(cnt)
            for g in range(7):
                for ch in groups[g]:
                    local[g, ch.L] += 1
            cnt = np.maximum(cnt, local)

        # pad each core to the common multiset; sort each group by L desc
        self.core_groups = []
        for c in range(NCORES):
            groups = per_core[c]
            padded = []
            for g in range(7):
                local = np.zeros(maxlen + 1, dtype=np.int64)
                for ch in groups[g]:
                    local[ch.L] += 1
                glist = list(groups[g])
                for L in range(1, maxlen + 1):
                    for _ in range(cnt[g, L] - local[L]):
                        glist.append(Chain(0, 0, L, g == 6, dummy=True))
                glist.sort(key=lambda ch: -ch.L)
                padded.append(glist)
            self.core_groups.append(padded)

        self.maxlen = maxlen
        self.Kmax = maxlen  # waves 0..Kmax-1
        # alive[g, k] = number of chains in group g with L > k  (same all cores)
        alive = np.zeros((7, maxlen), dtype=np.int64)
        for g in range(7):
            for L in range(1, maxlen + 1):
                alive[g, :L] += cnt[g, L]
        # group order in each wave: delta group (6) first, then 0..5
        self.gorder = [6, 0, 1, 2, 3, 4, 5]
        self.alive = alive
        self.m = [int(alive[:, k].sum()) for k in range(maxlen)]  # wave widths
        self.delta_w = int(alive[6, 0])  # width of the delta block in wave 0
        self.wave_off = np.concatenate([[0], np.cumsum(self.m)]).astype(np.int64)
        self.M_total = int(self.wave_off[-1])
        self.M0 = self.m[0]
        self.M_ge1 = self.M_total - self.M0

        # per-wave, per-group column offsets (within the wave), common
        self.goff = np.zeros((maxlen, 8), dtype=np.int64)
        for k in range(maxlen):
            off = 0
            for gi, g in enumerate(self.gorder):
                self.goff[k, gi] = off
                off += alive[g, k]
            self.goff[k, 7] = off

        # wave-0 alpha segments (common): [(alpha, start, end)] over cols,
        # delta block occupies [0, delta_w)
        self.w0_segs = []
        for gi, g in enumerate(self.gorder):
            if g == 6:
                continue
            s, e = int(self.goff[0, gi]), int(self.goff[0, gi + 1])
            if e > s:
                self.w0_segs.append((g, s, e))

        # segment copy ranges for wave k>=1: h_prev of wave k = for each group,
        # prefix of its block in wave k-1.  [(src_start, dst_start, width)]
        self.seg_copies = []
        for k in range(1, maxlen):
            segs = []
            for gi in range(7):
                w = int(self.alive[self.gorder[gi], k])
                if w > 0:
                    segs.append((int(self.goff[k - 1, gi]), int(self.goff[k, gi]), w))
            self.seg_copies.append(segs)

    def core_columns(self, c):
        """Yield (chain, k, col) for every real position of core c."""
        for k in range(self.Kmax):
            for gi, g in enumerate(self.gorder):
                base = int(self.wave_off[k] + self.goff[k, gi])
                glist = self.core_groups[c][g]
                # alive chains at depth k are the first alive[g,k] of the group
                for j in range(int(self.alive[g, k])):
                    ch = glist[j]
                    yield ch, k, base + j


def build_core_inputs(sched, c, x, a, reset, carry, initial_h, f32=np.float32):
    """Build the per-core DRAM input arrays (bf16 stored as uint16 view later)."""
    M_total, M0, M_ge1 = sched.M_total, sched.M0, sched.M_ge1
    dw = sched.delta_w
    n0 = c * NLOC

    xgt0 = np.zeros((D, M0), f32)            # wave-0 x columns
    xs6 = np.zeros((A, D, M_ge1), f32)       # pre-masked x for waves >= 1
    mask6 = np.zeros((A, M_total), f32)      # one-hot action per real column
    mrep = np.zeros((A, D, M_ge1), f32)      # replicated masks for waves >= 1
    mdelta = np.zeros((A, D, dw), f32)       # masks for the delta block (wave 0)
    c0t = np.tile(initial_h[:, None], (1, max(dw, 1)))[:, :dw].astype(f32)

    for ch, k, col in sched.core_columns(c):
        if ch.dummy:
            continue
        t = ch.t0 + k
        al = int(a[n0 + ch.n, t])
        mask6[al, col] = 1.0
        if k == 0:
            xgt0[:, col] = x[n0 + ch.n, t]
            if ch.is_delta:
                dcol = col  # delta block is first in wave 0
                assert dcol < dw
                mdelta[al, :, dcol] = 1.0
                c0t[:, dcol] = carry[n0 + ch.n, 0]
        else:
            cg = col - M0
            xs6[al, :, cg] = x[n0 + ch.n, t]
            mrep[al, :, cg] = 1.0
    return dict(xgt0=xgt0, xs6=xs6, mask6=mask6, mrep=mrep, mdelta=mdelta, c0t=c0t)


# ----------------------------------------------------------------------------
# Numpy simulation of the exact wave kernel (for logic validation)
# ----------------------------------------------------------------------------

def simulate_core(sched, inp, w_i, w_h, b, initial_h):
    """Runs the wave algorithm in numpy, mirroring the device dataflow."""
    M0, M_ge1 = sched.M0, sched.M_ge1
    dw = sched.delta_w
    h0 = initial_h.astype(np.float32)
    # U[alpha] = h0 @ w_h_zr[alpha]  (2H,)
    U = np.stack([h0 @ w_h[al, :, : 2 * H] for al in range(A)])  # [A, 2H]
    d0t = inp["c0t"] - h0[:, None]                               # [128, dw]

    y = np.zeros((H, sched.M_total), np.float32)
    hbufs = []
    for k in range(sched.Kmax):
        mk = sched.m[k]
        lo = int(sched.wave_off[k])
        mask6 = inp["mask6"][:, lo:lo + mk]                      # [A, mk]
        if k == 0:
            # zr psum: U-add + B-add + x-side + delta correction
            zr = np.zeros((2 * H, mk), np.float32)
            zr += U.T @ mask6                                    # U-add
            zr += b[:, : 2 * H].T @ mask6                        # B-add
            for al, s, e in sched.w0_segs:
                zr[:, s:e] += w_i[al, :, : 2 * H].T @ inp["xgt0"][:, s:e]
            for al in range(A):                                  # delta corr
                zr[:, :dw] += w_h[al, :, : 2 * H].T @ (d0t * inp["mdelta"][al])
                zr[:, :dw] += w_i[al, :, : 2 * H].T @ (inp["xgt0"][:, :dw] * inp["mdelta"][al])
            z = 1 / (1 + np.exp(-zr[:H]))
            r = 1 / (1 + np.exp(-zr[H:]))
            rh = r * h0[:, None]
            rh[:, :dw] = r[:, :dw] * inp["c0t"]
            cd = np.zeros((H, mk), np.float32)
            cd += b[:, 2 * H:].T @ mask6
            for al, s, e in sched.w0_segs:
                cd[:, s:e] += w_i[al, :, 2 * H:].T @ inp["xgt0"][:, s:e]
                cd[:, s:e] += w_h[al, :, 2 * H:].T @ rh[:, s:e]
            for al in range(A):
                cd[:, :dw] += w_h[al, :, 2 * H:].T @ (rh[:, :dw] * inp["mdelta"][al][:H])
                cd[:, :dw] += w_i[al, :, 2 * H:].T @ (inp["xgt0"][:, :dw] * inp["mdelta"][al])
            c = np.tanh(cd)
            hprev = np.tile(h0[:, None], (1, mk))
            hprev[:, :dw] = inp["c0t"]
        else:
            hprev = np.zeros((H, mk), np.float32)
            for src, dst, w in sched.seg_copies[k - 1]:
                hprev[:, dst:dst + w] = hbufs[k - 1][:, src:src + w]
            lo1 = lo - M0
            xs6 = inp["xs6"][:, :, lo1:lo1 + mk]
            mrep = inp["mrep"][:, :, lo1:lo1 + mk]
            ht = [hprev * mrep[al] for al in range(A)]
            zr = np.zeros((2 * H, mk), np.float32)
            zr += b[:, : 2 * H].T @ mask6
            for al in range(A):
                zr += w_i[al, :, : 2 * H].T @ xs6[al]
                zr += w_h[al, :, : 2 * H].T @ ht[al]
            z = 1 / (1 + np.exp(-zr[:H]))
            r = 1 / (1 + np.exp(-zr[H:]))
            cd = np.zeros((H, mk), np.float32)
            cd += b[:, 2 * H:].T @ mask6
            for al in range(A):
                cd += w_i[al, :, 2 * H:].T @ xs6[al]
                cd += w_h[al, :, 2 * H:].T @ (r * ht[al])
            c = np.tanh(cd)
        hnew = hprev + z * (c - hprev)
        hbufs.append(hnew)
        y[:, lo:lo + sched.m[k]] = hnew
    return y


def unpermute(sched, c, y):
    """y: [H, M_total] -> states for core c [NLOC, T, H]."""
    out = np.zeros((NLOC, T, H), np.float32)
    for ch, k, col in sched.core_columns(c):
        if ch.dummy:
            continue
        out[ch.n, ch.t0 + k] = y[:, col]
    return out


# ----------------------------------------------------------------------------
# Bass/Tile device kernel
# ----------------------------------------------------------------------------

def build_bass(sched):
    import concourse.bass as bass
    import concourse.bacc as bacc
    import concourse.tile as tile
    from concourse import mybir

    f32 = mybir.dt.float32
    bf16 = mybir.dt.bfloat16
    SIG = mybir.ActivationFunctionType.Sigmoid
    TANH = mybir.ActivationFunctionType.Tanh

    M_total, M0, M_ge1, dw = sched.M_total, sched.M0, sched.M_ge1, sched.delta_w
    Kmax = sched.Kmax
    woff = sched.wave_off

    nc = bacc.Bacc()
    # DRAM parameters (per-core data; one shared SPMD program)
    p_xgt0 = nc.declare_dram_parameter("xgt0", [D, M0], bf16, isOutput=False)
    p_xs6 = nc.declare_dram_parameter("xs6", [A, D, M_ge1], bf16, isOutput=False)
    p_mask6 = nc.declare_dram_parameter("mask6", [A, M_total], bf16, isOutput=False)
    p_mrep = nc.declare_dram_parameter("mrep", [A, D, M_ge1], bf16, isOutput=False)
    p_mdelta = nc.declare_dram_parameter("mdelta", [A, D, dw], bf16, isOutput=False)
    p_c0t = nc.declare_dram_parameter("c0t", [D, dw], f32, isOutput=False)
    p_h0 = nc.declare_dram_parameter("h0", [D, 1], f32, isOutput=False)
    p_wi = nc.declare_dram_parameter("wi", [A, D, G3], f32, isOutput=False)
    p_wh = nc.declare_dram_parameter("wh", [A, D, G3], f32, isOutput=False)
    p_b = nc.declare_dram_parameter("bb", [A, G3], f32, isOutput=False)
    p_y = nc.declare_dram_parameter("y", [D, M_total], bf16, isOutput=True)

    with bass.ExitStack() as ctx:
        tc = ctx.enter_context(tile.TileContext(nc))
        const = ctx.enter_context(tc.tile_pool(name="const", bufs=1))
        big = ctx.enter_context(tc.tile_pool(name="big", bufs=1))
        work = ctx.enter_context(tc.tile_pool(name="work", bufs=2))
        hp = ctx.enter_context(tc.tile_pool(name="hp", bufs=2))
        psum = ctx.enter_context(tc.tile_pool(name="psum", bufs=2, space="PSUM"))
        psu = ctx.enter_context(tc.tile_pool(name="psu", bufs=1, space="PSUM"))

        # ---- constants / weights ----
        wi_sb = const.tile([128, A, 3, 128], bf16)
        wh_sb = const.tile([128, A, 3, 128], bf16)
        nc.gpsimd.dma_start(out=wi_sb, in_=p_wi[:].rearrange("a d (k g) -> d a k g", k=3))
        nc.gpsimd.dma_start(out=wh_sb, in_=p_wh[:].rearrange("a d (k g) -> d a k g", k=3))
        b_f = const.tile([A, 3, 128], f32)
        nc.sync.dma_start(out=b_f, in_=p_b[:].rearrange("a (k g) -> a k g", k=3))
        b_sb = const.tile([A, 3, 128], bf16)
        nc.vector.tensor_copy(b_sb, b_f)
        h0f = const.tile([128, 1], f32)
        nc.sync.dma_start(out=h0f, in_=p_h0[:])
        h0b = const.tile([128, 1], bf16)
        nc.vector.tensor_copy(h0b, h0f)
        mask6_sb = const.tile([A, M_total], bf16)
        nc.sync.dma_start(out=mask6_sb, in_=p_mask6[:])

        # U[al] = h0 @ w_h_zr[al] -> one psum row; DRAM round-trip to get the
        # [A, 2, 128] partition layout; then UB = U + b (zr blocks), bf16.
        u_dram = nc.dram_tensor("u_scratch", [A, 2, 128], f32)
        u_row = const.tile([1, A, 2, 128], f32)
        u_row_flat = u_row.rearrange("p a k g -> p (a k g)")
        for i in range(3):
            pu = psu.tile([1, 512], f32, tag="pu", name="pu")
            for j in range(4):
                flat = i * 512 + j * 128
                al, blk = flat // 256, (flat // 128) % 2
                nc.tensor.matmul(pu[0:1, j * 128:(j + 1) * 128], h0b,
                                 wh_sb[:, al, blk, :], start=True, stop=True)
            nc.vector.tensor_copy(u_row_flat[:, i * 512:(i + 1) * 512], pu)
        nc.sync.dma_start(out=u_dram[:].rearrange("a k g -> (a k g)")[None, :],
                          in_=u_row)
        u6 = const.tile([A, 2, 128], f32)
        nc.sync.dma_start(out=u6, in_=u_dram[:])
        ub_sb = const.tile([A, 2, 128], bf16)
        nc.vector.tensor_add(ub_sb, u6, b_f[:, 0:2, :])

        # delta-block constants
        c0f = const.tile([128, max(dw, 1)], f32)
        c0b = const.tile([128, max(dw, 1)], bf16)
        d0m = const.tile([128, A, max(dw, 1)], bf16)   # (c0-h0) masked per action
        x0m = const.tile([128, A, max(dw, 1)], bf16)   # x[:, :dw] masked per action
        mdelta_sb = const.tile([128, A, max(dw, 1)], bf16)
        if dw > 0:
            nc.sync.dma_start(out=c0f, in_=p_c0t[:])
            nc.vector.tensor_copy(c0b, c0f)
            nc.sync.dma_start(out=mdelta_sb, in_=p_mdelta[:].rearrange("a d c -> d a c"))
            d0f = const.tile([128, max(dw, 1)], f32)
            nc.vector.tensor_scalar_sub(d0f, c0f, h0f)
            d0b = const.tile([128, max(dw, 1)], bf16)
            nc.vector.tensor_copy(d0b, d0f)
            for al in range(A):
                nc.vector.tensor_mul(d0m[:, al, :], d0b, mdelta_sb[:, al, :])

        # ---- big streams ----
        xgt0_sb = big.tile([128, M0], bf16)
        nc.sync.dma_start(out=xgt0_sb, in_=p_xgt0[:])
        if dw > 0:
            for al in range(A):
                nc.vector.tensor_mul(x0m[:, al, :], xgt0_sb[:, 0:dw], mdelta_sb[:, al, :])
        xs6_sb = big.tile([128, A, M_ge1], bf16)
        mrep_sb = big.tile([128, A, M_ge1], bf16)
        for al in range(A):
            nc.sync.dma_start(out=xs6_sb[:, al, :], in_=p_xs6[al])
            nc.sync.dma_start(out=mrep_sb[:, al, :], in_=p_mrep[al])

        hbufs = []

        def gru_tile(k, lo, tw, hprev_tile, ht6, segs):
            """Emit one column tile [lo, lo+tw) of wave k.

            hprev_tile: [128, tw] bf16 or None (wave 0: h_prev = h0/c0)
            ht6: [128, A, tw] masked h_prev (waves >= 1) or None
            segs: list of (al, s, e) tile-local maskless segments (wave 0)
            """
            first = (k == 0 and lo == 0)
            glo = woff[k] + lo  # global column
            clo = glo - M0      # column in the >=1 arrays
            m6 = mask6_sb[:, glo:glo + tw]

            zps = [psum.tile([128, tw], f32, tag="zrz", name="zrz"),
                   psum.tile([128, tw], f32, tag="zrr", name="zrr")]
            for blk in range(2):
                zp = zps[blk]
                mms = []
                if k == 0:
                    mms.append((ub_sb[:, blk, :], m6, None))
                    for al, s, e in segs:
                        mms.append((wi_sb[:, al, blk, :], xgt0_sb[:, glo + s:glo + e], (s, e)))
                    if first and dw > 0:
                        for al in range(A):
                            mms.append((wh_sb[:, al, blk, :], d0m[:, al, :], (0, dw)))
                            mms.append((wi_sb[:, al, blk, :], x0m[:, al, :], (0, dw)))
                else:
                    mms.append((b_sb[:, blk, :], m6, None))
                    for al in range(A):
                        mms.append((wi_sb[:, al, blk, :], xs6_sb[:, al, clo:clo + tw], None))
                    for al in range(A):
                        mms.append((wh_sb[:, al, blk, :], ht6[:, al, :], None))
                for i, (lhsT, rhs, rng) in enumerate(mms):
                    out = zp if rng is None else zp[:, rng[0]:rng[1]]
                    nc.tensor.matmul(out, lhsT, rhs, start=(i == 0),
                                     stop=(i == len(mms) - 1), skip_group_check=True)

            zb = work.tile([128, tw], bf16, tag="z", name="zb")
            rb = work.tile([128, tw], bf16, tag="r", name="rb")
            nc.scalar.activation(out=zb, in_=zps[0], func=SIG)
            nc.scalar.activation(out=rb, in_=zps[1], func=SIG)

            # rh (wave 0) or rh6 (waves >= 1)
            cp = psum.tile([128, tw], f32, tag="cand", name="cp")
            mms = []
            if k == 0:
                rh = work.tile([128, tw], bf16, tag="rh", name="rh")
                nc.vector.tensor_scalar_mul(rh, rb, h0f)
                if first and dw > 0:
                    nc.vector.tensor_mul(rh[:, 0:dw], rb[:, 0:dw], c0b)
                mms.append((b_sb[:, 2, :], m6, None))
                for al, s, e in segs:
                    mms.append((wi_sb[:, al, 2, :], xgt0_sb[:, glo + s:glo + e], (s, e)))
                    mms.append((wh_sb[:, al, 2, :], rh[:, s:e], (s, e)))
                if first and dw > 0:
                    rhd = work.tile([128, A, max(dw, 1)], bf16, tag="rhd", name="rhd")
                    for al in range(A):
                        nc.vector.tensor_mul(rhd[:, al, :], rh[:, 0:dw], mdelta_sb[:, al, :])
                    for al in range(A):
                        mms.append((wh_sb[:, al, 2, :], rhd[:, al, :], (0, dw)))
                        mms.append((wi_sb[:, al, 2, :], x0m[:, al, :], (0, dw)))
            else:
                rh6 = work.tile([128, A, tw], bf16, tag="rh6", name="rh6")
                for al in range(A):
                    nc.vector.tensor_mul(rh6[:, al, :], rb, ht6[:, al, :])
                mms.append((b_sb[:, 2, :], m6, None))
                for al in range(A):
                    mms.append((wi_sb[:, al, 2, :], xs6_sb[:, al, clo:clo + tw], None))
                for al in range(A):
                    mms.append((wh_sb[:, al, 2, :], rh6[:, al, :], None))
            for i, (lhsT, rhs, rng) in enumerate(mms):
                out = cp if rng is None else cp[:, rng[0]:rng[1]]
                nc.tensor.matmul(out, lhsT, rhs, start=(i == 0),
                                 stop=(i == len(mms) - 1), skip_group_check=True)

            cb = work.tile([128, tw], bf16, tag="c", name="cb")
            nc.scalar.activation(out=cb, in_=cp, func=TANH)

            # h_new = h_prev + z * (c - h_prev)
            db = work.tile([128, tw], bf16, tag="d", name="db")
            eb = work.tile([128, tw], bf16, tag="e", name="eb")
            hout = hbufs[k]
            if k == 0:
                nc.vector.tensor_scalar_sub(db, cb, h0f)
                if first and dw > 0:
                    nc.vector.tensor_sub(db[:, 0:dw], cb[:, 0:dw], c0b)
                nc.vector.tensor_mul(eb, zb, db)
                nc.vector.tensor_scalar_add(hout[:, lo:lo + tw], eb, h0f)
                if first and dw > 0:
                    nc.vector.tensor_add(hout[:, 0:dw], eb[:, 0:dw], c0b)
            else:
                nc.vector.tensor_sub(db, cb, hprev_tile)
                nc.vector.tensor_mul(eb, zb, db)
                nc.vector.tensor_add(hout[:, lo:lo + tw], hprev_tile, eb)

        # ---- wave loop ----
        for k in range(Kmax):
            mk = sched.m[k]
            hb = big.tile([128, mk], bf16, tag=f"hbuf{k}", name=f"hbuf{k}")
            hbufs.append(hb)
            if k == 0:
                hprev = None
            else:
                hprev = hp.tile([128, mk], bf16, tag="hprev", name="hprev")
                for src, dst, w in sched.seg_copies[k - 1]:
                    nc.vector.tensor_copy(hprev[:, dst:dst + w],
                                          hbufs[k - 1][:, src:src + w])
            for lo in range(0, mk, PSUM_TILE):
                tw = min(PSUM_TILE, mk - lo)
                if k == 0:
                    segs = []
                    for al, s, e in sched.w0_segs:
                        s2, e2 = max(s, lo), min(e, lo + tw)
                        if e2 > s2:
                            segs.append((al, s2 - lo, e2 - lo))
                    gru_tile(0, lo, tw, None, None, segs)
                else:
                    ht6 = work.tile([128, A, tw], bf16, tag="ht6", name="ht6")
                    for al in range(A):
                        nc.vector.tensor_mul(
                            ht6[:, al, :], hprev[:, lo:lo + tw],
                            mrep_sb[:, al, woff[k] - M0 + lo: woff[k] - M0 + lo + tw])
                    gru_tile(k, lo, tw, hprev[:, lo:lo + tw], ht6, None)
            nc.sync.dma_start(out=p_y[:, woff[k]:woff[k] + mk], in_=hbufs[k])

    nc.compile()
    return nc


def run_device(sched, core_inputs, w_i, w_h, b, initial_h, trace=False):
    from concourse.bass_utils import run_bass_kernel_spmd
    import ml_dtypes

    bf = ml_dtypes.bfloat16
    nc = build_bass(sched)
    in_maps = []
    for ci in core_inputs:
        in_maps.append({
            "xgt0": ci["xgt0"].astype(bf),
            "xs6": ci["xs6"].astype(bf),
            "mask6": ci["mask6"].astype(bf),
            "mrep": ci["mrep"].astype(bf),
            "mdelta": ci["mdelta"].astype(bf),
            "c0t": ci["c0t"].astype(np.float32),
            "h0": initial_h.reshape(D, 1).astype(np.float32),
            "wi": w_i.astype(np.float32),
            "wh": w_h.astype(np.float32),
            "bb": b.astype(np.float32),
        })
    res = run_bass_kernel_spmd(nc, in_maps, list(range(NCORES)), trace=trace)
    ys = [np.asarray(r["y"]).astype(np.float32) for r in res.results]
    return ys, res


# ----------------------------------------------------------------------------
# Entry point
# ----------------------------------------------------------------------------

def kernel(x, a, reset, carry, w_i, w_h, b, initial_h):
    x = np.asarray(x, np.float32)
    a = np.asarray(a)
    reset = np.asarray(reset)
    carry = np.asarray(carry, np.float32)
    w_i = np.asarray(w_i, np.float32)
    w_h = np.asarray(w_h, np.float32)
    b = np.asarray(b, np.float32)
    initial_h = np.asarray(initial_h, np.float32)

    sched = Schedule(reset, a)
    core_inputs = [build_core_inputs(sched, c, x, a, reset, carry, initial_h)
                   for c in range(NCORES)]
    ys, _ = run_device(sched, core_inputs, w_i, w_h, b, initial_h)
    states = np.zeros((N, T, H), np.float32)
    for c in range(NCORES):
        states[c * NLOC:(c + 1) * NLOC] = unpermute(sched, c, ys[c])
    return states, states, initial_h[None, :]
